# revision 1
# baseline (speedup 1.0000x reference)
"""Trainium2 Bass kernel for ItemEmbeddingLayer (embedding_lookup).

Reference computation:
    out = Q_matrix[items] @ skill_embedding[user]      # [8192, 128] f32

Sharding (per the hint): the single active user's embedding row
(skill_embedding[user], [256,128]) is replicated to all 8 cores; `items`
is sharded batch-wise, 1024 per core; Q_matrix is replicated (each core
gathers only the rows its items need via indirect DMA).

Per-core device kernel:
  1. 8x indirect_dma_start gathers pull the 1024 needed Q rows (bf16 —
     exact, Q is binary) into [item, skill] SBUF tiles, 1 row/partition.
  2. PE transposes (with a bf16 identity) flip each [128,128] block into
     the [skill, item] layout matmul weights need; DVE/ACT copy back.
  3. skill_embedding row is split on-device into bf16 hi + lo parts
     (emb ~= hi + lo), recovering ~fp32 precision from bf16 matmuls.
  4. 8 l-chunks x (2 s-chunks x {hi,lo}) matmuls accumulate in fp32 PSUM.
  5. PSUM -> SBUF copies, one 512KB DMA out.
"""

import numpy as np
import ml_dtypes

import concourse.bass as bass
import concourse.bacc as bacc
import concourse.mybir as mybir
from concourse.tile import TileContext
from concourse.bass_utils import run_bass_kernel_spmd

N_CORES = 8
L = 8192          # total items (seq len)
LC = L // N_CORES # items per core
S = 256           # skills
K = 128           # hidden
R = 4096          # Q_matrix rows (n items vocab)
P = 128           # partitions
NCH = LC // P     # l-chunks per core


def build_bass() -> bass.Bass:
    nc = bacc.Bacc(trn_type="TRN2", dynamic_dma_scratch_size=131072)
    q = nc.declare_dram_parameter("q_bf16", [R, S], mybir.dt.bfloat16, isOutput=False)
    idx = nc.declare_dram_parameter("idx", [P, NCH], mybir.dt.int32, isOutput=False)
    emb = nc.declare_dram_parameter("emb", [S, K], mybir.dt.float32, isOutput=False)
    ident = nc.declare_dram_parameter("ident", [P, P], mybir.dt.bfloat16, isOutput=False)
    out = nc.declare_dram_parameter("out", [LC, K], mybir.dt.float32, isOutput=True)

    with (
        TileContext(nc) as tc,
        tc.tile_pool(name="main", bufs=1) as pool,
        tc.tile_pool(name="gat", bufs=4) as gpool,
        tc.tile_pool(name="tps", bufs=4, space="PSUM") as tpsum,
        tc.tile_pool(name="acc", bufs=4, space="PSUM") as apsum,
    ):
        idx_t = pool.tile([P, NCH], mybir.dt.int32)
        nc.sync.dma_start(out=idx_t[:], in_=idx[:])
        ident_t = pool.tile([P, P], mybir.dt.bfloat16)
        nc.sync.dma_start(out=ident_t[:], in_=ident[:])

        emb_t = pool.tile([P, 2, K], mybir.dt.float32)
        nc.sync.dma_start(out=emb_t[:], in_=emb[:].rearrange("(e p) k -> p e k", p=P))

        # emb = hi + lo with both parts bf16; products accumulate in fp32
        # PSUM, so two bf16 passes recover ~16 mantissa bits of emb.
        # hilo[:, e, :] = [hi_e | lo_e] so one N=256 matmul does both passes.
        hilo = pool.tile([P, 2, 2 * K], mybir.dt.bfloat16)
        nc.vector.tensor_copy(hilo[:, :, 0:K], emb_t[:])
        hi32 = pool.tile([P, 2, K], mybir.dt.float32)
        nc.vector.tensor_copy(hi32[:], hilo[:, :, 0:K])
        nc.vector.tensor_sub(hilo[:, :, K : 2 * K], emb_t[:], hi32[:])

        for c in range(NCH):
            # q_sb[p, s] = Q[idx[p, c], s] = Q[items[c*128 + p], s]
            q_sb = gpool.tile([P, S], mybir.dt.bfloat16, tag="q_sb")
            nc.gpsimd.indirect_dma_start(
                out=q_sb[:],
                out_offset=None,
                in_=q[:],
                in_offset=bass.IndirectOffsetOnAxis(ap=idx_t[:, c : c + 1], axis=0),
            )
            qT = gpool.tile([P, 2, P], mybir.dt.bfloat16, tag="qT")
            for e in range(2):
                tp = tpsum.tile([P, P], mybir.dt.bfloat16, tag="tp")
                nc.tensor.transpose(
                    out=tp[:], in_=q_sb[:, e * P : (e + 1) * P], identity=ident_t[:]
                )
                # alternate copy engine so DVE and ACT share the load
                if e == 0:
                    nc.vector.tensor_copy(qT[:, e, :], tp[:])
                else:
                    nc.scalar.copy(qT[:, e, :], tp[:])

            # ps[:, :K] = q@hi, ps[:, K:] = q@lo (e-sum via PSUM accumulate)
            ps = apsum.tile([P, 2 * K], mybir.dt.float32, tag="ps")
            for e in range(2):
                nc.tensor.matmul(
                    ps[:], qT[:, e, :], hilo[:, e, :],
                    start=(e == 0), stop=(e == 1),
                )
            o = gpool.tile([P, K], mybir.dt.float32, tag="o")
            nc.scalar.copy(o[:], ps[:, 0:K])
            nc.vector.tensor_add(o[:], o[:], ps[:, K : 2 * K])
            nc.sync.dma_start(out=out[c * P : (c + 1) * P, :], in_=o[:])

    nc.compile()
    return nc


_CACHE: dict = {}


def get_nc() -> bass.Bass:
    if "nc" not in _CACHE:
        _CACHE["nc"] = build_bass()
    return _CACHE["nc"]


def make_in_maps(user, Q_matrix, items, skill_embedding):
    user = int(np.asarray(user))
    Q = np.asarray(Q_matrix, dtype=np.float32)
    items = np.asarray(items).astype(np.int64)
    emb = np.ascontiguousarray(np.asarray(skill_embedding)[user], dtype=np.float32)
    q_bf = Q.astype(ml_dtypes.bfloat16)  # exact: Q is 0/1
    ident = np.eye(P, dtype=ml_dtypes.bfloat16)

    in_maps = []
    for i in range(N_CORES):
        it = items[i * LC : (i + 1) * LC].astype(np.int32)
        # indirect gather c pulls row idx[p, c] into partition p
        idx_arr = np.ascontiguousarray(it.reshape(NCH, P).T)  # [128, NCH]
        in_maps.append({"q_bf16": q_bf, "idx": idx_arr, "emb": emb, "ident": ident})
    return in_maps


def kernel(user, Q_matrix, items, skill_embedding, _trace=False, _result_box=None):
    in_maps = make_in_maps(user, Q_matrix, items, skill_embedding)
    res = run_bass_kernel_spmd(get_nc(), in_maps, list(range(N_CORES)), trace=_trace)
    if _result_box is not None:
        _result_box.append(res)
    out = np.concatenate([res.results[i]["out"] for i in range(N_CORES)], axis=0)
    return np.ascontiguousarray(out, dtype=np.float32)

